# revision 26
# baseline (speedup 1.0000x reference)
"""GAT message-passing kernel for Trainium2 (8 NeuronCores, SPMD).

Problem (per full input):
    B=8, S=512, N=32 neighbors, H=256, V=100001
    out[b,s,:] = sum_n softmax_n(leakyrelu(a_w . [src, cand_n]) + mask*NEG) * cand_n
    candidates = [self] + 32 neighbors (self never masked)

Sharding: data-parallel over B — core c handles batch row c with a
replicated embedding table.

Per-core algorithm (s-tiles of 128 nodes, 4 tiles):
    - candidate id tile [128, 33] (col 0 = self, 1..32 = neighbors)
    - one indirect DMA per candidate slot (HW allows one offset per
      partition per instruction) gathers 128 rows of 256 f32 into
      F[s][n,h]; 132 gathers/core dominate the runtime (~1.4us each)
    - logits fused behind each gather: scalar_tensor_tensor computes
      sum_h F[s,n,h]*awc[h] into z[:,n] via accum_out (one DVE op/slot)
    - deferred softmax: per slot group, e = exp(leakyrelu(z)+mask*NEG)
      without max subtraction (logits are tiny; clamped at 80), so the
      TensorE aggregation sum_n diag(e_n) @ F_n accumulates in PSUM
      while later slots are still gathering; the 1/sum(e) normalization
      is folded into the PSUM-evacuation scale on ScalarE
"""

import numpy as np

B, S, N, H, V = 8, 512, 32, 256, 100001
NC1 = N + 1  # 33 candidate slots (self + neighbors)
P = 128
S_TILES = S // P
NEG = -1.0e9
SLOPE = 0.2
N_CORES = 8

# Tuning knobs
F_BUFS = 4  # gather-tile buffering
GROUPS = [(0, 11), (11, 22), (22, 29), (29, 32), (32, 33)]  # slot groups for the deferred softmax

_CACHE: dict = {}


def _build_nc():
    import concourse.bacc as bacc
    import concourse.mybir as mybir
    import concourse.tile as tile
    from concourse import bass
    from concourse.masks import make_identity

    f32 = mybir.dt.float32
    i32 = mybir.dt.int32
    Alu = mybir.AluOpType
    Act = mybir.ActivationFunctionType
    X = mybir.AxisListType.X

    nc = bacc.Bacc(
        "TRN2",
        target_bir_lowering=False,
        debug=False,
        enable_asserts=False,
        num_devices=N_CORES,
    )

    ids_d = nc.dram_tensor("node_ids", [S, 1], i32, kind="ExternalInput").ap()
    nbr_d = nc.dram_tensor("neighs", [S, N], i32, kind="ExternalInput").ap()
    msk_d = nc.dram_tensor("mask", [S, N], i32, kind="ExternalInput").ap()
    emb_d = nc.dram_tensor("emb_table", [V, H], f32, kind="ExternalInput").ap()
    aw_d = nc.dram_tensor("a_w", [2, H], f32, kind="ExternalInput").ap()
    ab_d = nc.dram_tensor("a_b", [1, 1], f32, kind="ExternalInput").ap()
    out_d = nc.dram_tensor("out", [S, H], f32, kind="ExternalOutput").ap()

    with tile.TileContext(nc) as tc:
        with (
            tc.tile_pool(name="cpool", bufs=1) as cpool,
            tc.tile_pool(name="fpool", bufs=F_BUFS) as fpool,
            tc.tile_pool(name="spool", bufs=2) as spool,
            tc.tile_pool(name="dpool", bufs=8) as dpool,
            tc.tile_pool(name="ppool", bufs=3, space="PSUM") as ppool,
        ):
            # ---- index/mask loads first so tile-0 gathers start ASAP ----
            idx_tiles = []
            mask_tiles = []
            for t in range(S_TILES):
                rows = slice(t * P, (t + 1) * P)
                idx = cpool.tile([P, NC1], i32, name=f"idx{t}")
                nc.sync.dma_start(out=idx[:, 0:1], in_=ids_d[rows, :])
                nc.sync.dma_start(out=idx[:, 1:NC1], in_=nbr_d[rows, :])
                mask_i = cpool.tile([P, N], i32, name=f"mask{t}")
                nc.sync.dma_start(out=mask_i[:], in_=msk_d[rows, :])
                idx_tiles.append(idx)
                mask_tiles.append(mask_i)

            # ---- constants (once) ----
            ident = cpool.tile([P, P], f32)
            make_identity(nc, ident)

            # replicate a_w rows (a_src = row 0, a_cand = row 1) to all
            # 128 partitions with a tiny indirect gather
            # one offset (row 0) per partition; 512 contiguous floats cover
            # both a_w rows -> [aws | awc] per partition
            aw_rep = cpool.tile([P, 2 * H], f32)
            nc.sync.dma_start(
                out=aw_rep[:],
                in_=aw_d.rearrange("a h -> (a h)").unsqueeze(0).to_broadcast([P, 2 * H]),
            )
            aws_rep = aw_rep[:, 0:H]
            awc_rep = aw_rep[:, H : 2 * H]

            ab_rep = cpool.tile([P, 1], f32)
            nc.sync.dma_start(out=ab_rep[:], in_=ab_d.to_broadcast([P, 1]))

            for t in range(S_TILES):
                rows = slice(t * P, (t + 1) * P)
                idx = idx_tiles[t]
                mask_f = spool.tile([P, N], f32)
                nc.vector.tensor_copy(mask_f[:], mask_tiles[t][:])

                F = fpool.tile([P, NC1 * H], f32)
                F3 = F.rearrange("p (n h) -> p n h", n=NC1)
                trash = spool.tile([P, H], f32)
                zsrc = spool.tile([P, 1], f32)
                z = spool.tile([P, NC1], f32)
                e = spool.tile([P, NC1], f32)
                deng = spool.tile([P, len(GROUPS)], f32)
                acc = ppool.tile([P, H], f32)

                # Normalization is deferred: per slot group, compute
                # eg = exp(leakyrelu(z)+mask*NEG) (z is tiny for this data, so
                # no max subtraction; clamp guards overflow) and accumulate
                # diag(eg_n) @ F_n into PSUM while later slots still gather.
                # The 1/sum(e) lands as a scale on the PSUM evacuation.
                for gi, (a, b) in enumerate(GROUPS):
                    for n in range(a, b):
                        nc.gpsimd.indirect_dma_start(
                            out=F3[:, n, :],
                            out_offset=None,
                            in_=emb_d,
                            in_offset=bass.IndirectOffsetOnAxis(
                                ap=idx[:, n : n + 1], axis=0
                            ),
                        )
                        if n == 0:
                            trash2 = spool.tile([P, H], f32)
                            nc.vector.scalar_tensor_tensor(
                                out=trash2[:],
                                in0=F3[:, 0, :],
                                scalar=1.0,
                                in1=aws_rep,
                                op0=Alu.mult,
                                op1=Alu.mult,
                                accum_out=zsrc[:],
                            )
                            nc.vector.tensor_scalar_add(
                                zsrc[:], zsrc[:], ab_rep[:]
                            )
                        nc.vector.scalar_tensor_tensor(
                            out=trash[:],
                            in0=F3[:, n, :],
                            scalar=1.0,
                            in1=awc_rep,
                            op0=Alu.mult,
                            op1=Alu.mult,
                            accum_out=z[:, n : n + 1],
                        )
                    zg = z[:, a:b]
                    nc.vector.tensor_scalar_add(zg, zg, zsrc[:])
                    # leakyrelu = max(x, 0.2x)
                    z2 = spool.tile([P, NC1], f32)
                    nc.vector.tensor_scalar_mul(z2[:, a:b], zg, SLOPE)
                    nc.vector.tensor_max(zg, zg, z2[:, a:b])
                    # neighbor masking (slot 0 = self, never masked)
                    ma, mb = max(a, 1), b
                    nc.vector.scalar_tensor_tensor(
                        out=z[:, ma:mb],
                        in0=mask_f[:, ma - 1 : mb - 1],
                        scalar=NEG,
                        in1=z[:, ma:mb],
                        op0=Alu.mult,
                        op1=Alu.add,
                    )
                    nc.vector.tensor_scalar_min(zg, zg, 80.0)
                    nc.scalar.activation(
                        e[:, a:b],
                        zg,
                        Act.Exp,
                        accum_out=deng[:, gi : gi + 1],
                    )
                    for n in range(a, b):
                        dg = dpool.tile([P, P], f32, name="dg")
                        if t == S_TILES - 1:
                            # keep DVE free for the tail's logit chain
                            nc.scalar.mul(dg[:], ident[:], e[:, n : n + 1])
                        else:
                            nc.vector.tensor_scalar_mul(
                                dg[:], ident[:], e[:, n : n + 1]
                            )
                        nc.tensor.matmul(
                            out=acc[:],
                            lhsT=dg[:],
                            rhs=F3[:, n, :],
                            start=(n == 0),
                            stop=(n == NC1 - 1),
                        )

                den = spool.tile([P, 1], f32)
                nc.vector.tensor_reduce(den[:], deng[:], axis=X, op=Alu.add)
                rden = spool.tile([P, 1], f32)
                nc.vector.reciprocal(rden[:], den[:])
                o = spool.tile([P, H], f32)
                nc.scalar.mul(o[:], acc[:], rden[:])
                nc.sync.dma_start(out=out_d[rows, :], in_=o[:])

    nc.compile()
    return nc


def _get_nc():
    if "nc" not in _CACHE:
        _CACHE["nc"] = _build_nc()
    return _CACHE["nc"]


def _ensure_axon_hooks():
    """Provide antenv.axon_hooks if the image lacks it, so trace=True /
    BASS_TRACE=1 profiling requests don't crash run_bass_kernel_spmd."""
    import sys
    import types

    try:
        import antenv.axon_hooks  # noqa: F401

        return
    except ImportError:
        pass
    try:
        import antenv
    except ImportError:
        return
    mod = types.ModuleType("antenv.axon_hooks")
    state = {"hook": None}

    def set_axon_ntff_profile_hook(h):
        state["hook"] = h

    def get_axon_ntff_profile_hook():
        if state["hook"] is None:
            try:
                from trn_agent_boot.trn_boot import _ntff_profile_via_ctypes

                state["hook"] = _ntff_profile_via_ctypes("/opt/axon/libaxon_pjrt.so")
            except Exception:
                return None
        return state["hook"]

    mod.set_axon_ntff_profile_hook = set_axon_ntff_profile_hook
    mod.get_axon_ntff_profile_hook = get_axon_ntff_profile_hook
    sys.modules["antenv.axon_hooks"] = mod
    antenv.axon_hooks = mod


def kernel(**inputs) -> np.ndarray:
    _ensure_axon_hooks()
    from concourse.bass_utils import run_bass_kernel_spmd

    node_ids = np.ascontiguousarray(
        np.asarray(inputs["node_ids"]).astype(np.int32).reshape(B, S, 1)
    )
    neighs = np.ascontiguousarray(
        np.asarray(inputs["neighs"]).astype(np.int32).reshape(B, S, N)
    )
    mask = np.ascontiguousarray(
        np.asarray(inputs["mask"]).astype(np.int32).reshape(B, S, N)
    )
    emb = np.ascontiguousarray(np.asarray(inputs["emb_table"], dtype=np.float32))
    a_w = np.ascontiguousarray(
        np.asarray(inputs["a_w"], dtype=np.float32).reshape(2, H)
    )
    a_b = np.ascontiguousarray(
        np.asarray(inputs["a_b"], dtype=np.float32).reshape(1, 1)
    )

    nc = _get_nc()
    in_maps = [
        {
            "node_ids": node_ids[c],
            "neighs": neighs[c],
            "mask": mask[c],
            "emb_table": emb,
            "a_w": a_w,
            "a_b": a_b,
        }
        for c in range(N_CORES)
    ]
    core_ids = list(range(N_CORES))
    try:
        res = run_bass_kernel_spmd(nc, in_maps, core_ids=core_ids)
    except Exception:
        # transient device wedge — retry once
        res = run_bass_kernel_spmd(nc, in_maps, core_ids=core_ids)
    _CACHE["last_res"] = res
    out = np.stack([res.results[c]["out"] for c in range(N_CORES)], axis=0)
    return out.astype(np.float32)


# revision 27
# speedup vs baseline: 1.0244x; 1.0244x over previous
"""GAT message-passing kernel for Trainium2 (8 NeuronCores, SPMD).

Problem (per full input):
    B=8, S=512, N=32 neighbors, H=256, V=100001
    out[b,s,:] = sum_n softmax_n(leakyrelu(a_w . [src, cand_n]) + mask*NEG) * cand_n
    candidates = [self] + 32 neighbors (self never masked)

Sharding: data-parallel over B — core c handles batch row c with a
replicated embedding table.

Per-core algorithm (s-tiles of 128 nodes, 4 tiles):
    - candidate id tile [128, 33] (col 0 = self, 1..32 = neighbors)
    - one indirect DMA per candidate slot (HW allows one offset per
      partition per instruction) gathers 128 rows of 256 f32 into
      F[s][n,h]; 132 gathers/core dominate the runtime (~1.4us each)
    - logits fused behind each gather: scalar_tensor_tensor computes
      sum_h F[s,n,h]*awc[h] into z[:,n] via accum_out (one DVE op/slot)
    - deferred softmax: per slot group, e = exp(leakyrelu(z)+mask*NEG)
      without max subtraction (logits are tiny; clamped at 80), so the
      TensorE aggregation sum_n diag(e_n) @ F_n accumulates in PSUM
      while later slots are still gathering; the 1/sum(e) normalization
      is folded into the PSUM-evacuation scale on ScalarE
"""

import numpy as np

B, S, N, H, V = 8, 512, 32, 256, 100001
NC1 = N + 1  # 33 candidate slots (self + neighbors)
P = 128
S_TILES = S // P
NEG = -1.0e9
SLOPE = 0.2
N_CORES = 8

# Tuning knobs
F_BUFS = 4  # gather-tile buffering
GROUPS = [(0, 11), (11, 22), (22, 31), (31, 33)]  # slot groups for the deferred softmax

_CACHE: dict = {}


def _build_nc():
    import concourse.bacc as bacc
    import concourse.mybir as mybir
    import concourse.tile as tile
    from concourse import bass
    from concourse.masks import make_identity

    f32 = mybir.dt.float32
    i32 = mybir.dt.int32
    Alu = mybir.AluOpType
    Act = mybir.ActivationFunctionType
    X = mybir.AxisListType.X

    nc = bacc.Bacc(
        "TRN2",
        target_bir_lowering=False,
        debug=False,
        enable_asserts=False,
        num_devices=N_CORES,
    )

    ids_d = nc.dram_tensor("node_ids", [S, 1], i32, kind="ExternalInput").ap()
    nbr_d = nc.dram_tensor("neighs", [S, N], i32, kind="ExternalInput").ap()
    msk_d = nc.dram_tensor("mask", [S, N], i32, kind="ExternalInput").ap()
    emb_d = nc.dram_tensor("emb_table", [V, H], f32, kind="ExternalInput").ap()
    aw_d = nc.dram_tensor("a_w", [2, H], f32, kind="ExternalInput").ap()
    ab_d = nc.dram_tensor("a_b", [1, 1], f32, kind="ExternalInput").ap()
    out_d = nc.dram_tensor("out", [S, H], f32, kind="ExternalOutput").ap()

    with tile.TileContext(nc) as tc:
        with (
            tc.tile_pool(name="cpool", bufs=1) as cpool,
            tc.tile_pool(name="fpool", bufs=F_BUFS) as fpool,
            tc.tile_pool(name="spool", bufs=2) as spool,
            tc.tile_pool(name="dpool", bufs=8) as dpool,
            tc.tile_pool(name="ppool", bufs=3, space="PSUM") as ppool,
        ):
            # ---- index/mask loads first so tile-0 gathers start ASAP ----
            idx_tiles = []
            mask_tiles = []
            for t in range(S_TILES):
                rows = slice(t * P, (t + 1) * P)
                idx = cpool.tile([P, NC1], i32, name=f"idx{t}")
                nc.sync.dma_start(out=idx[:, 0:1], in_=ids_d[rows, :])
                nc.sync.dma_start(out=idx[:, 1:NC1], in_=nbr_d[rows, :])
                mask_i = cpool.tile([P, N], i32, name=f"mask{t}")
                nc.sync.dma_start(out=mask_i[:], in_=msk_d[rows, :])
                idx_tiles.append(idx)
                mask_tiles.append(mask_i)

            # ---- constants (once) ----
            ident = cpool.tile([P, P], f32)
            make_identity(nc, ident)

            # replicate a_w rows (a_src = row 0, a_cand = row 1) to all
            # 128 partitions with a tiny indirect gather
            # one offset (row 0) per partition; 512 contiguous floats cover
            # both a_w rows -> [aws | awc] per partition
            aw_rep = cpool.tile([P, 2 * H], f32)
            nc.sync.dma_start(
                out=aw_rep[:],
                in_=aw_d.rearrange("a h -> (a h)").unsqueeze(0).to_broadcast([P, 2 * H]),
            )
            aws_rep = aw_rep[:, 0:H]
            awc_rep = aw_rep[:, H : 2 * H]

            ab_rep = cpool.tile([P, 1], f32)
            nc.sync.dma_start(out=ab_rep[:], in_=ab_d.to_broadcast([P, 1]))

            for t in range(S_TILES):
                rows = slice(t * P, (t + 1) * P)
                idx = idx_tiles[t]
                mask_f = spool.tile([P, N], f32)
                nc.vector.tensor_copy(mask_f[:], mask_tiles[t][:])

                F = fpool.tile([P, NC1 * H], f32)
                F3 = F.rearrange("p (n h) -> p n h", n=NC1)
                trash = spool.tile([P, H], f32)
                zsrc = spool.tile([P, 1], f32)
                z = spool.tile([P, NC1], f32)
                e = spool.tile([P, NC1], f32)
                deng = spool.tile([P, len(GROUPS)], f32)
                acc = ppool.tile([P, H], f32)

                # Normalization is deferred: per slot group, compute
                # eg = exp(leakyrelu(z)+mask*NEG) (z is tiny for this data, so
                # no max subtraction; clamp guards overflow) and accumulate
                # diag(eg_n) @ F_n into PSUM while later slots still gather.
                # The 1/sum(e) lands as a scale on the PSUM evacuation.
                for gi, (a, b) in enumerate(GROUPS):
                    for n in range(a, b):
                        nc.gpsimd.indirect_dma_start(
                            out=F3[:, n, :],
                            out_offset=None,
                            in_=emb_d,
                            in_offset=bass.IndirectOffsetOnAxis(
                                ap=idx[:, n : n + 1], axis=0
                            ),
                        )
                        if n == 0:
                            trash2 = spool.tile([P, H], f32)
                            nc.vector.scalar_tensor_tensor(
                                out=trash2[:],
                                in0=F3[:, 0, :],
                                scalar=1.0,
                                in1=aws_rep,
                                op0=Alu.mult,
                                op1=Alu.mult,
                                accum_out=zsrc[:],
                            )
                            nc.vector.tensor_scalar_add(
                                zsrc[:], zsrc[:], ab_rep[:]
                            )
                        nc.vector.scalar_tensor_tensor(
                            out=trash[:],
                            in0=F3[:, n, :],
                            scalar=1.0,
                            in1=awc_rep,
                            op0=Alu.mult,
                            op1=Alu.mult,
                            accum_out=z[:, n : n + 1],
                        )
                    zg = z[:, a:b]
                    nc.vector.tensor_scalar_add(zg, zg, zsrc[:])
                    # leakyrelu = max(x, 0.2x)
                    z2 = spool.tile([P, NC1], f32)
                    nc.vector.tensor_scalar_mul(z2[:, a:b], zg, SLOPE)
                    nc.vector.tensor_max(zg, zg, z2[:, a:b])
                    # neighbor masking (slot 0 = self, never masked)
                    ma, mb = max(a, 1), b
                    nc.vector.scalar_tensor_tensor(
                        out=z[:, ma:mb],
                        in0=mask_f[:, ma - 1 : mb - 1],
                        scalar=NEG,
                        in1=z[:, ma:mb],
                        op0=Alu.mult,
                        op1=Alu.add,
                    )
                    nc.vector.tensor_scalar_min(zg, zg, 80.0)
                    nc.scalar.activation(
                        e[:, a:b],
                        zg,
                        Act.Exp,
                        accum_out=deng[:, gi : gi + 1],
                    )
                    for n in range(a, b):
                        dg = dpool.tile([P, P], f32, name="dg")
                        nc.vector.tensor_scalar_mul(
                            dg[:], ident[:], e[:, n : n + 1]
                        )
                        nc.tensor.matmul(
                            out=acc[:],
                            lhsT=dg[:],
                            rhs=F3[:, n, :],
                            start=(n == 0),
                            stop=(n == NC1 - 1),
                        )

                den = spool.tile([P, 1], f32)
                nc.vector.tensor_reduce(den[:], deng[:], axis=X, op=Alu.add)
                rden = spool.tile([P, 1], f32)
                nc.vector.reciprocal(rden[:], den[:])
                o = spool.tile([P, H], f32)
                nc.scalar.mul(o[:], acc[:], rden[:])
                nc.sync.dma_start(out=out_d[rows, :], in_=o[:])

    nc.compile()
    return nc


def _get_nc():
    if "nc" not in _CACHE:
        _CACHE["nc"] = _build_nc()
    return _CACHE["nc"]


def _ensure_axon_hooks():
    """Provide antenv.axon_hooks if the image lacks it, so trace=True /
    BASS_TRACE=1 profiling requests don't crash run_bass_kernel_spmd."""
    import sys
    import types

    try:
        import antenv.axon_hooks  # noqa: F401

        return
    except ImportError:
        pass
    try:
        import antenv
    except ImportError:
        return
    mod = types.ModuleType("antenv.axon_hooks")
    state = {"hook": None}

    def set_axon_ntff_profile_hook(h):
        state["hook"] = h

    def get_axon_ntff_profile_hook():
        if state["hook"] is None:
            try:
                from trn_agent_boot.trn_boot import _ntff_profile_via_ctypes

                state["hook"] = _ntff_profile_via_ctypes("/opt/axon/libaxon_pjrt.so")
            except Exception:
                return None
        return state["hook"]

    mod.set_axon_ntff_profile_hook = set_axon_ntff_profile_hook
    mod.get_axon_ntff_profile_hook = get_axon_ntff_profile_hook
    sys.modules["antenv.axon_hooks"] = mod
    antenv.axon_hooks = mod


def kernel(**inputs) -> np.ndarray:
    _ensure_axon_hooks()
    from concourse.bass_utils import run_bass_kernel_spmd

    node_ids = np.ascontiguousarray(
        np.asarray(inputs["node_ids"]).astype(np.int32).reshape(B, S, 1)
    )
    neighs = np.ascontiguousarray(
        np.asarray(inputs["neighs"]).astype(np.int32).reshape(B, S, N)
    )
    mask = np.ascontiguousarray(
        np.asarray(inputs["mask"]).astype(np.int32).reshape(B, S, N)
    )
    emb = np.ascontiguousarray(np.asarray(inputs["emb_table"], dtype=np.float32))
    a_w = np.ascontiguousarray(
        np.asarray(inputs["a_w"], dtype=np.float32).reshape(2, H)
    )
    a_b = np.ascontiguousarray(
        np.asarray(inputs["a_b"], dtype=np.float32).reshape(1, 1)
    )

    nc = _get_nc()
    in_maps = [
        {
            "node_ids": node_ids[c],
            "neighs": neighs[c],
            "mask": mask[c],
            "emb_table": emb,
            "a_w": a_w,
            "a_b": a_b,
        }
        for c in range(N_CORES)
    ]
    core_ids = list(range(N_CORES))
    try:
        res = run_bass_kernel_spmd(nc, in_maps, core_ids=core_ids)
    except Exception:
        # transient device wedge — retry once
        res = run_bass_kernel_spmd(nc, in_maps, core_ids=core_ids)
    _CACHE["last_res"] = res
    out = np.stack([res.results[c]["out"] for c in range(N_CORES)], axis=0)
    return out.astype(np.float32)
